# revision 13
# baseline (speedup 1.0000x reference)
"""Trainium2 Bass kernel for nn_AttentionBlock (B=8, H=W=32, C=512, 8 heads).

Data-parallel over batch: each of 8 NeuronCores does one batch element.

The kernel is organized around keeping ScalarE (softmax exp, the serial
floor at ~59us of ACT work) busy while TensorE work rides in its shadow:

  ramp:    x^T loaded directly via XBAR transpose-DMA (bf16), weights
           bf16 via a second queue; q^T/k^T for head pair 0; v s-tiles
           0-1; exp-table warm-up.
  phase 2: head pairs p=0..3, j-major slots (j, g): the two heads' S^T
           s-tile matmuls run concurrently in PE row groups 0-1/2-3
           (K=64 each) -> one ScalarE exp(N=1024) -> same-pair PV
           matmuls lagging one slot.  Remaining v s-tiles, later pairs'
           q^T/k^T projections, and the entire out projection are woven
           into the S^T PSUM ring as small background items.
  tail:    per t-tile 4-7: single c-tile-3 matmul + add to the SBUF
           partial -> DMA.

All matmul operands are bf16 (fp32 PSUM accumulation, fp32 softmax
denominators/reciprocals); rel err vs the fp32 reference ~2e-3, well
inside the 2e-2 gate.  Softmax without max-subtraction (logits ~N(0,1)),
denominators via a ones-column appended to V, v-bias folded into the
output bias host-side.
"""

import math
import os
from contextlib import ExitStack

import numpy as np

import concourse.bass as bass
import concourse.mybir as mybir
import concourse.tile as tile
from concourse import bacc

T = 1024          # tokens per batch element (32*32)
C = 512           # channels
HEADS = 8
HC = C // HEADS   # 64
P = 128           # partitions
NT = T // P       # 8 t-tiles
NCT = C // P      # 4 c-tiles
CHUNK = 512       # PSUM bank = 512 fp32
NCH = T // CHUNK  # 2 chunks
F32 = mybir.dt.float32
BF16 = mybir.dt.bfloat16
EXP_SCALE = 1.0 / math.sqrt(HC)
VW = HC + 1       # v channels + ones column
NPAIR = HEADS // 2


def build_program(debug_dumps: bool = False):
    nc = bacc.Bacc("TRN2", num_devices=8, debug=False)

    x_d = nc.dram_tensor("x", [T, C], BF16, kind="ExternalInput")
    wqkv_d = nc.dram_tensor("qkv_w", [C, 3 * C], BF16, kind="ExternalInput")
    wout_d = nc.dram_tensor("out_w", [C, C], BF16, kind="ExternalInput")
    qkb_d = nc.dram_tensor("qk_b", [2 * C], F32, kind="ExternalInput")
    ob_d = nc.dram_tensor("out_b", [C], F32, kind="ExternalInput")
    out_d = nc.dram_tensor("out", [T, C], F32, kind="ExternalOutput")

    with tile.TileContext(nc) as tc, ExitStack() as ctx:
        # ---------------- SBUF pools ----------------
        const = ctx.enter_context(tc.tile_pool(name="const", bufs=1))
        persist = ctx.enter_context(tc.tile_pool(name="persist", bufs=1))
        workp = ctx.enter_context(tc.tile_pool(name="workp", bufs=6))
        otp = ctx.enter_context(tc.tile_pool(name="otp", bufs=4))
        partp = ctx.enter_context(tc.tile_pool(name="partp", bufs=1))

        # exp table warm-up ASAP (the ~2.7us table load hides under DMA)
        warm = const.tile([1, 2], F32, tag="warm", name="warm")
        nc.gpsimd.memset(warm[:, 0:1], 0.0)
        nc.scalar.activation(warm[:, 1:2], warm[:, 0:1],
                             mybir.ActivationFunctionType.Exp)

        # x tiles then PE transposes (XBAR transpose-DMA measured ~8us/tile
        # -- far slower than the PE path)
        xT = [persist.tile([P, T], BF16, tag=f"xT{m}", name=f"xT{m}")
              for m in range(NCT)]
        identity = const.tile([P, P], BF16, tag="ident", name="ident")
        from concourse.masks import make_identity
        make_identity(nc, identity[:])
        xin_cm = tc.tile_pool(name="xin", bufs=1)
        xin = xin_cm.__enter__()
        # x pairs on 4 distinct DMA queues; pairs 0-1 (t 0:512) are the
        # critical set for the first S^T slot, so they get the engines whose
        # issue queues are empty at program start
        xpair = []
        xq = (nc.sync, nc.scalar, nc.gpsimd, nc.sync)
        for ip in range(NT // 2):
            xt_in = xin.tile([P, 2 * C], BF16, tag=f"xin{ip}", name=f"xin{ip}")
            xq[ip].dma_start(xt_in[:].rearrange("p (u c) -> p u c", u=2),
                             x_d.ap()[2 * ip * P:(2 * ip + 2) * P, :]
                             .rearrange("(u p) c -> p u c", p=P))
            xpair.append(xt_in)
        xts = [xpair[i // 2][:, (i % 2) * C:(i % 2 + 1) * C]
               for i in range(NT)]

        # weights: pair-0 q/k columns first, one queue per c-tile so the
        # 4 critical 640-col chunks interleave with the x pairs
        wq = []  # [c-tile][128, 1536] (q | k | v)
        for m in range(NCT):
            t_ = persist.tile([P, 3 * C], BF16, tag=f"wq{m}", name=f"wq{m}")
            xq[m].dma_start(t_[:, 0:640],
                            wqkv_d.ap()[m * P:(m + 1) * P, 0:640])
            wq.append(t_)

        # biases; column m of qkb_all = qkv_b[128m:128m+128]
        qkb_all = const.tile([P, 2 * C // P], F32, tag="qkball", name="qkb_all")
        nc.gpsimd.dma_start(
            qkb_all[:], qkb_d.ap().rearrange("(m p) -> p m", p=P)
        )
        qkb_t = [qkb_all[:, m:m + 1] for m in range(2 * C // P)]

        # non-critical weight tails stay off the vector queue (it has the
        # transpose copies early) and spread over sync/scalar/gpsimd
        wq_tail_q = (nc.sync, nc.scalar, nc.gpsimd, nc.scalar)
        for m in range(NCT):
            wq_tail_q[m].dma_start(wq[m][:, 640:3 * C],
                                   wqkv_d.ap()[m * P:(m + 1) * P, 640:3 * C])
        wo = []  # [c-tile][128, 512]
        wo_q = (nc.sync, nc.scalar, nc.gpsimd, nc.sync)
        for m in range(NCT):
            t_ = persist.tile([P, C], BF16, tag=f"wo{m}", name=f"wo{m}")
            wo_q[m].dma_start(t_[:], wout_d.ap()[m * P:(m + 1) * P, :])
            wo.append(t_)

        ones8 = const.tile([P, HEADS, 1], F32, tag="ones8", name="ones8")
        nc.gpsimd.memset(ones8[:], 1.0)
        ob_row = const.tile([1, C], F32, tag="obrow", name="ob_row")
        nc.sync.dma_start(ob_row[:], ob_d.ap().rearrange("(o c) -> o c", o=1))
        obb = const.tile([P, C], F32, tag="obb", name="obb")
        nc.gpsimd.partition_broadcast(obb[:], ob_row[:], channels=P)

        VAW = HEADS * VW + (P - VW)  # 128-wide lhsT reads stay in-tile
        vaug = [persist.tile([P, VAW], BF16, tag=f"va{i}", name=f"va{i}")
                for i in range(NT)]
        qkT = [persist.tile([P, T], BF16, tag=f"qk{m}", name=f"qk{m}")
               for m in range(NCT)]
        # compact k^T per pair: head 2p in rows 0:64, head 2p+1 in 64:128
        kTc = [persist.tile([P, T], BF16, tag=f"kc{m}", name=f"kc{m}")
               for m in range(NCT)]
        anorm = [persist.tile([P, T], BF16, tag=f"an{m}", name=f"an{m}")
                 for m in range(NCT)]
        # fp32 out-projection partials (c-tiles 0-2 + bias) for t-tiles 4-7
        part = [partp.tile([P, C], F32, tag=f"pt{i}", name=f"pt{i}")
                for i in range(4)]
        # pair-0 j=0 exp slots live until its PV runs during j=1
        exh0 = [persist.tile([P, 2 * CHUNK], BF16, tag=f"ex0{g}",
                             name=f"ex0{g}") for g in range(NT)]

        # ================= background item emitters =================
        def emit_qk_half(m, j, pool):
            """qkv-projection m-tile, chunk j (q: m<4 -> qkT; k: -> kTc)."""
            ps_qk = pool.tile([P, 2 * CHUNK], F32, tag="st", name="ps_bg")
            js = slice(j * CHUNK, (j + 1) * CHUNK)
            for cc in range(NCT):
                nc.tensor.matmul(
                    ps_qk[:, 0:CHUNK],
                    wq[cc][:, m * P:(m + 1) * P],
                    xT[cc][:, js],
                    start=(cc == 0),
                    stop=(cc == NCT - 1),
                )
            dst = qkT[m] if m < NCT else kTc[m - NCT]
            nc.vector.tensor_scalar_add(dst[:, js], ps_qk[:, 0:CHUNK],
                                        qkb_t[m][:])

        def emit_v_2tiles(i0, pool):
            """vaug[i0], vaug[i0+1]: [128(t), 8, 65], [:, h, 64] = 1.0"""
            ps_v = pool.tile([P, 2 * CHUNK], F32, tag="st", name="ps_v")
            for ii in range(2):
                i = i0 + ii
                for m in range(NCT):
                    nc.tensor.matmul(
                        ps_v[:, ii * CHUNK:(ii + 1) * CHUNK],
                        xT[m][:, i * P:(i + 1) * P],
                        wq[m][:, 2 * C:3 * C],
                        start=(m == 0),
                        stop=(m == NCT - 1),
                    )
            for ii in range(2):
                i = i0 + ii
                va3 = vaug[i][:, 0:HEADS * VW].rearrange(
                    "p (h d) -> p h d", d=VW)
                nc.vector.tensor_copy(
                    va3[:, :, 0:HC],
                    ps_v[:, ii * CHUNK:(ii + 1) * CHUNK].rearrange(
                        "p (h d) -> p h d", h=HEADS),
                )
                nc.vector.tensor_copy(va3[:, :, HC:VW], ones8[:])

        def emit_outproj_partial(i, pool):
            """part[i-4] = out-proj c-tiles 0-2 + bias for t-tile i."""
            ps_o = pool.tile([P, 2 * CHUNK], F32, tag="st", name="ps_op")
            for cc in range(NCT - 1):
                nc.tensor.matmul(
                    ps_o[:, 0:CHUNK],
                    anorm[cc][:, i * P:(i + 1) * P],
                    wo[cc][:],
                    start=(cc == 0),
                    stop=(cc == NCT - 2),
                )
            nc.vector.tensor_tensor(
                part[i - 4][:], ps_o[:, 0:CHUNK], obb[:],
                op=mybir.AluOpType.add,
            )

        def emit_outproj_full(i, pool):
            """out rows [128*i, 128*(i+1)): all c-tiles + bias -> DMA."""
            ps_o = pool.tile([P, 2 * CHUNK], F32, tag="st", name="ps_of")
            for cc in range(NCT):
                nc.tensor.matmul(
                    ps_o[:, 0:CHUNK],
                    anorm[cc][:, i * P:(i + 1) * P],
                    wo[cc][:],
                    start=(cc == 0),
                    stop=(cc == NCT - 1),
                )
            ot = otp.tile([P, C], F32, tag="ot", name="ot")
            nc.vector.tensor_tensor(
                ot[:], ps_o[:, 0:CHUNK], obb[:], op=mybir.AluOpType.add,
            )
            nc.sync.dma_start(out_d.ap()[i * P:(i + 1) * P, :], ot[:])

        # ================= ramp =================
        ps1_cm = tc.tile_pool(name="ps1", bufs=2, space="PSUM")
        ps1 = ps1_cm.__enter__()
        # x PE transpose (bf16); xT[m] = x^T rows [128m,128m+128) [c, t]
        for i in range(NT):
            ps_tr = ps1.tile([P, C], BF16, tag="tr", name="ps_tr")
            for m in range(NCT):
                nc.tensor.transpose(
                    ps_tr[:, m * P:(m + 1) * P],
                    xts[i][:, m * P:(m + 1) * P],
                    identity[:],
                )
            for m in range(NCT):
                nc.vector.tensor_copy(
                    xT[m][:, i * P:(i + 1) * P], ps_tr[:, m * P:(m + 1) * P]
                )
        # only the j=0 halves of pair-0's q^T/k^T gate the first exp; their
        # j=1 halves and all of v are woven into pair-0's j=0 slots
        emit_qk_half(0, 0, ps1)             # q pair 0, t 0:512
        emit_qk_half(NCT, 0, ps1)           # k pair 0, s 0:512
        ps1_cm.__exit__(None, None, None)
        xin_cm.__exit__(None, None, None)

        # vaug junk-region zero (keeps 128-wide lhsT reads NaN-free);
        # emitted after the ramp so DVE's early queue stays clear -- only
        # needed before the first PV
        for i in range(NT):
            nc.vector.tensor_scalar_mul(
                vaug[i][:, HEADS * VW:VAW], wq[0][:, 0:VAW - HEADS * VW], 0.0
            )

        # ================= phase 2: attention (head pairs) =================
        # weave schedule: (p, j, g) -> emitter run after that slot
        WEAVE = {
            (0, 0, 0): lambda pool: emit_qk_half(0, 1, pool),
            (0, 0, 1): lambda pool: emit_qk_half(NCT, 1, pool),
            (0, 0, 2): lambda pool: emit_v_2tiles(0, pool),
            (0, 0, 3): lambda pool: emit_v_2tiles(2, pool),
            (0, 0, 4): lambda pool: emit_v_2tiles(4, pool),
            (0, 0, 5): lambda pool: emit_v_2tiles(6, pool),
            (0, 1, 1): lambda pool: emit_qk_half(1, 0, pool),
            (0, 1, 3): lambda pool: emit_qk_half(1, 1, pool),
            (0, 1, 5): lambda pool: emit_qk_half(NCT + 1, 0, pool),
            (0, 1, 6): lambda pool: emit_qk_half(NCT + 1, 1, pool),
            (1, 0, 2): lambda pool: emit_qk_half(2, 0, pool),
            (1, 0, 5): lambda pool: emit_qk_half(2, 1, pool),
            (1, 1, 2): lambda pool: emit_qk_half(NCT + 2, 0, pool),
            (1, 1, 5): lambda pool: emit_qk_half(NCT + 2, 1, pool),
            (2, 0, 2): lambda pool: emit_qk_half(3, 0, pool),
            (2, 0, 5): lambda pool: emit_qk_half(3, 1, pool),
            (2, 1, 2): lambda pool: emit_qk_half(NCT + 3, 0, pool),
            (2, 1, 5): lambda pool: emit_qk_half(NCT + 3, 1, pool),
            (3, 0, 1): lambda pool: emit_outproj_partial(4, pool),
            (3, 0, 3): lambda pool: emit_outproj_partial(5, pool),
            (3, 0, 5): lambda pool: emit_outproj_partial(6, pool),
            (3, 0, 6): lambda pool: emit_outproj_partial(7, pool),
            (3, 1, 3): lambda pool: emit_outproj_full(0, pool),
            (3, 1, 5): lambda pool: emit_outproj_full(1, pool),
            # full(2)/full(3) are emitted after the j=1 loop: their matmuls
            # have no dependency on the final normalize, so they execute
            # right behind the last PV and keep the PE warm through it
        }

        with (
            tc.tile_pool(name="expsp", bufs=6) as expsp,
            tc.tile_pool(name="ps_st", bufs=2, space="PSUM") as ps_st,
            tc.tile_pool(name="ps_pv", bufs=2, space="PSUM") as ps_pv,
        ):
            def emit_pv_pair(p, exs, pvt, g):
                """PV matmuls for s-tile g, both heads of pair p, one j."""
                for hh in range(2):
                    nc.tensor.matmul(
                        pvt[:, hh * CHUNK:(hh + 1) * CHUNK],
                        vaug[g][:, (2 * p + hh) * VW:(2 * p + hh) * VW + P],
                        exs[:, hh * CHUNK:(hh + 1) * CHUNK],
                        start=(g == 0),
                        stop=(g == NT - 1),
                    )

            def emit_normalize(p, j, pvt):
                """Scale both heads' PV by 1/denominator -> anorm[p]."""
                js = slice(j * CHUNK, (j + 1) * CHUNK)
                # NB: reciprocal_approx_fast reading PSUM directly returns
                # garbage on HW (sim-only OK) -- stage through SBUF.
                dtmp = workp.tile([1, 2 * CHUNK], F32, tag="dtmp", name="dtmp")
                recip = workp.tile([1, 2 * CHUNK], F32, tag="recip",
                                   name="recip")
                nc.vector.tensor_copy(dtmp[:], pvt[HC:HC + 1, :])
                nc.vector.reciprocal_approx_fast(recip[:], dtmp[:])
                bcast = workp.tile([HC, 2 * CHUNK], F32, tag="bcast",
                                   name="bcast")
                nc.gpsimd.partition_broadcast(bcast[:], recip[:],
                                              channels=HC)
                bsrc = bcast
                for hh in range(2):
                    nc.vector.tensor_tensor(
                        anorm[p][hh * HC:(hh + 1) * HC, js],
                        pvt[0:HC, hh * CHUNK:(hh + 1) * CHUNK],
                        bsrc[0:HC, hh * CHUNK:(hh + 1) * CHUNK],
                        op=mybir.AluOpType.mult,
                    )

            ex0slots = []
            for p in range(NPAIR):
                pvt = {}
                for j in range(NCH):
                    pvt[j] = ps_pv.tile([P, 2 * CHUNK], F32, tag="pv",
                                        name="pv")
                    exslots = []
                    for g in range(NT):
                        st_ps = ps_st.tile([P, 2 * CHUNK], F32, tag="st",
                                           name="st")
                        # two heads' S^T concurrently in row groups 0-1/2-3
                        for hh in range(2):
                            hlo = hh * HC
                            nc.tensor.matmul(
                                st_ps[:, hh * CHUNK:(hh + 1) * CHUNK],
                                kTc[p][hlo:hlo + HC, g * P:(g + 1) * P],
                                qkT[p][hlo:hlo + HC,
                                       j * CHUNK:(j + 1) * CHUNK],
                                start=True,
                                stop=True,
                            )
                        if p == 0 and j == 0:
                            exs = exh0[g]
                        else:
                            exs = expsp.tile([P, 2 * CHUNK], BF16, tag="exh",
                                             name="exh")
                        exslots.append(exs)
                        nc.scalar.activation(
                            exs[:],
                            st_ps[:],
                            mybir.ActivationFunctionType.Exp,
                            scale=EXP_SCALE,
                        )
                        if p == 0 and j == 0:
                            pass  # PV deferred to j=1 (v weaves in here)
                        elif p == 0 and j == 1:
                            emit_pv_pair(0, ex0slots[g], pvt[0], g)
                            if g >= 1:
                                emit_pv_pair(0, exslots[g - 1], pvt[1], g - 1)
                        elif g >= 1:
                            emit_pv_pair(p, exslots[g - 1], pvt[j], g - 1)
                        if (p, j, g) in WEAVE:
                            WEAVE[(p, j, g)](ps_st)
                    if p == 0 and j == 0:
                        ex0slots = exslots
                    elif p == 0 and j == 1:
                        emit_pv_pair(0, exslots[NT - 1], pvt[1], NT - 1)
                        emit_normalize(0, 0, pvt[0])
                        emit_normalize(0, 1, pvt[1])
                    else:
                        emit_pv_pair(p, exslots[NT - 1], pvt[j], NT - 1)
                        if not (p == NPAIR - 1 and j == 1):
                            emit_normalize(p, j, pvt[j])

                if p == NPAIR - 1:
                    # ---- tail: final normalize (j=1) fused with the last
                    # t-tiles' out-proj.  Quarter-pipelined: denominator
                    # copies ride the now-idle ACT engine ('copy' is in the
                    # exp table set -- no table reload), recip/mults on DVE,
                    # broadcast on GpSimd, then per t-tile c3-matmul -> add
                    # (DVE/GpSimd alternating) -> DMA.
                    emit_outproj_full(2, ps_st)
                    emit_outproj_full(3, ps_st)
                    pvl = pvt[1]
                    Q = CHUNK // 2
                    den3 = pvl[HC:HC + 1, :].rearrange(
                        "p (h q) -> p h q", h=2)
                    dts, rcs, bcs = [], [], []
                    for u in range(2):
                        dt_ = workp.tile([1, 2 * Q], F32, tag="dtmp",
                                         name=f"dtf{u}")
                        nc.scalar.activation(
                            dt_[:].rearrange("p (h q) -> p h q", h=2),
                            den3[:, :, u * Q:(u + 1) * Q],
                            mybir.ActivationFunctionType.Copy,
                        )
                        dts.append(dt_)
                    for u in range(2):
                        rc = workp.tile([1, 2 * Q], F32, tag="recip",
                                        name=f"rcf{u}")
                        nc.vector.reciprocal_approx_fast(rc[:], dts[u][:])
                        rcs.append(rc)
                        bc = workp.tile([HC, 2 * Q], F32, tag="bcast",
                                        name=f"bcf{u}")
                        nc.gpsimd.partition_broadcast(bc[:], rc[:],
                                                      channels=HC)
                        bcs.append(bc)
                    for u in range(2):
                        for hh in range(2):
                            nc.vector.tensor_tensor(
                                anorm[NPAIR - 1][hh * HC:(hh + 1) * HC,
                                                 CHUNK + u * Q:
                                                 CHUNK + (u + 1) * Q],
                                pvl[0:HC, hh * CHUNK + u * Q:
                                    hh * CHUNK + (u + 1) * Q],
                                bcs[u][0:HC, hh * Q:(hh + 1) * Q],
                                op=mybir.AluOpType.mult,
                            )
                        for i in (4 + 2 * u, 5 + 2 * u):
                            pool2 = (ps_pv, ps_st)[i % 2]
                            ps_o2 = pool2.tile([P, 2 * CHUNK], F32,
                                               tag="pv" if i % 2 == 0
                                               else "st",
                                               name="ps_tl")
                            nc.tensor.matmul(
                                ps_o2[:, 0:CHUNK],
                                anorm[NCT - 1][:, i * P:(i + 1) * P],
                                wo[NCT - 1][:],
                                start=True,
                                stop=True,
                            )
                            ot = otp.tile([P, C], F32, tag="ot", name="ot")
                            nc.vector.tensor_tensor(
                                ot[:],
                                ps_o2[:, 0:CHUNK],
                                part[i - 4][:],
                                op=mybir.AluOpType.add,
                            )
                            dma_eng = (nc.sync, nc.scalar)[i % 2]
                            dma_eng.dma_start(
                                out_d.ap()[i * P:(i + 1) * P, :],
                                ot[:],
                            )

    nc.compile()
    return nc


def host_prep(x, qkv_w, qkv_b, out_w, out_b):
    """Host-side input prep shared by kernel() and the sim harness."""
    import ml_dtypes

    x = np.asarray(x)
    B = x.shape[0]
    x2 = x.reshape(B, T, C).astype(np.float32)
    wq2 = np.asarray(qkv_w).reshape(C, 3 * C).astype(np.float32)
    wo2 = np.asarray(out_w).reshape(C, C).astype(np.float32)
    qkv_b = np.asarray(qkv_b).astype(np.float32)
    out_b = np.asarray(out_b).astype(np.float32)

    bf = ml_dtypes.bfloat16
    x_bf = x2.astype(bf)
    wq_bf = wq2.astype(bf)
    wo_bf = wo2.astype(bf)
    # fold the v-bias through the output projection (A_norm += b_v shifts
    # out by b_v @ W_out)
    b_v = qkv_b[2 * C:3 * C]
    ob_eff = (
        out_b.astype(np.float64)
        + b_v.astype(np.float64) @ wo_bf.astype(np.float64)
    ).astype(np.float32)
    qkb = np.ascontiguousarray(qkv_b[0:2 * C])
    return x_bf, wq_bf, wo_bf, qkb, ob_eff


_CACHED_NC = None


def _get_nc():
    global _CACHED_NC
    if _CACHED_NC is None:
        _CACHED_NC = build_program()
    return _CACHED_NC


def kernel(x, qkv_w, qkv_b, out_w, out_b):
    """Full inputs in, full output out.  Shards batch across 8 NeuronCores."""
    from concourse.bass_utils import run_bass_kernel_spmd

    x = np.asarray(x)
    B, H, W, Cc = x.shape
    assert (B, H, W, Cc) == (8, 32, 32, C)
    x_bf, wq_bf, wo_bf, qkb, ob_eff = host_prep(x, qkv_w, qkv_b, out_w, out_b)

    nc = _get_nc()
    in_maps = [
        {
            "x": np.ascontiguousarray(x_bf[b]),
            "qkv_w": np.ascontiguousarray(wq_bf),
            "out_w": np.ascontiguousarray(wo_bf),
            "qk_b": qkb,
            "out_b": ob_eff,
        }
        for b in range(B)
    ]
    trace = bool(int(os.environ.get("KERNEL_TRACE", "0")))
    res = run_bass_kernel_spmd(nc, in_maps, core_ids=list(range(B)), trace=trace)
    if trace and res.exec_time_ns is not None:
        print(f"HW exec time: {res.exec_time_ns} ns")
    kernel.last_results = res
    out = np.stack([res.results[b]["out"] for b in range(B)], axis=0)
    return out.reshape(B, H, W, Cc)


kernel.last_results = None



# revision 15
# speedup vs baseline: 1.0058x; 1.0058x over previous
"""Trainium2 Bass kernel for nn_AttentionBlock (B=8, H=W=32, C=512, 8 heads).

Data-parallel over batch: each of 8 NeuronCores does one batch element.

The kernel is organized around keeping ScalarE (softmax exp, the serial
floor at ~59us of ACT work) busy while TensorE work rides in its shadow:

  ramp:    x^T loaded directly via XBAR transpose-DMA (bf16), weights
           bf16 via a second queue; q^T/k^T for head pair 0; v s-tiles
           0-1; exp-table warm-up.
  phase 2: head pairs p=0..3, j-major slots (j, g): the two heads' S^T
           s-tile matmuls run concurrently in PE row groups 0-1/2-3
           (K=64 each) -> one ScalarE exp(N=1024) -> same-pair PV
           matmuls lagging one slot.  Remaining v s-tiles, later pairs'
           q^T/k^T projections, and the entire out projection are woven
           into the S^T PSUM ring as small background items.
  tail:    per t-tile 4-7: single c-tile-3 matmul + add to the SBUF
           partial -> DMA.

All matmul operands are bf16 (fp32 PSUM accumulation, fp32 softmax
denominators/reciprocals); rel err vs the fp32 reference ~2e-3, well
inside the 2e-2 gate.  Softmax without max-subtraction (logits ~N(0,1)),
denominators via a ones-column appended to V, v-bias folded into the
output bias host-side.
"""

import math
import os
from contextlib import ExitStack

import numpy as np

import concourse.bass as bass
import concourse.mybir as mybir
import concourse.tile as tile
from concourse import bacc

T = 1024          # tokens per batch element (32*32)
C = 512           # channels
HEADS = 8
HC = C // HEADS   # 64
P = 128           # partitions
NT = T // P       # 8 t-tiles
NCT = C // P      # 4 c-tiles
CHUNK = 512       # PSUM bank = 512 fp32
NCH = T // CHUNK  # 2 chunks
F32 = mybir.dt.float32
BF16 = mybir.dt.bfloat16
EXP_SCALE = 1.0 / math.sqrt(HC)
VW = HC + 1       # v channels + ones column
NPAIR = HEADS // 2


def build_program(debug_dumps: bool = False):
    nc = bacc.Bacc("TRN2", num_devices=8, debug=False)

    x_d = nc.dram_tensor("x", [T, C], BF16, kind="ExternalInput")
    wqkv_d = nc.dram_tensor("qkv_w", [C, 3 * C], BF16, kind="ExternalInput")
    wout_d = nc.dram_tensor("out_w", [C, C], BF16, kind="ExternalInput")
    qkb_d = nc.dram_tensor("qk_b", [2 * C], F32, kind="ExternalInput")
    ob_d = nc.dram_tensor("out_b", [C], F32, kind="ExternalInput")
    out_d = nc.dram_tensor("out", [T, C], F32, kind="ExternalOutput")

    with tile.TileContext(nc) as tc, ExitStack() as ctx:
        # ---------------- SBUF pools ----------------
        const = ctx.enter_context(tc.tile_pool(name="const", bufs=1))
        persist = ctx.enter_context(tc.tile_pool(name="persist", bufs=1))
        workp = ctx.enter_context(tc.tile_pool(name="workp", bufs=6))
        otp = ctx.enter_context(tc.tile_pool(name="otp", bufs=4))
        partp = ctx.enter_context(tc.tile_pool(name="partp", bufs=1))

        # exp table warm-up ASAP (the ~2.7us table load hides under DMA)
        warm = const.tile([1, 2], F32, tag="warm", name="warm")
        nc.gpsimd.memset(warm[:, 0:1], 0.0)
        nc.scalar.activation(warm[:, 1:2], warm[:, 0:1],
                             mybir.ActivationFunctionType.Exp)

        # x tiles then PE transposes (XBAR transpose-DMA measured ~8us/tile
        # -- far slower than the PE path)
        xT = [persist.tile([P, T], BF16, tag=f"xT{m}", name=f"xT{m}")
              for m in range(NCT)]
        identity = const.tile([P, P], BF16, tag="ident", name="ident")
        from concourse.masks import make_identity
        make_identity(nc, identity[:])
        xin_cm = tc.tile_pool(name="xin", bufs=1)
        xin = xin_cm.__enter__()
        # x pairs on 4 distinct DMA queues; pairs 0-1 (t 0:512) are the
        # critical set for the first S^T slot, so they get the engines whose
        # issue queues are empty at program start
        xpair = []
        xq = (nc.sync, nc.scalar, nc.gpsimd, nc.sync)
        for ip in range(NT // 2):
            xt_in = xin.tile([P, 2 * C], BF16, tag=f"xin{ip}", name=f"xin{ip}")
            xq[ip].dma_start(xt_in[:].rearrange("p (u c) -> p u c", u=2),
                             x_d.ap()[2 * ip * P:(2 * ip + 2) * P, :]
                             .rearrange("(u p) c -> p u c", p=P))
            xpair.append(xt_in)
        xts = [xpair[i // 2][:, (i % 2) * C:(i % 2 + 1) * C]
               for i in range(NT)]

        # weights: pair-0 q/k columns first, one queue per c-tile so the
        # 4 critical 640-col chunks interleave with the x pairs
        wq = []  # [c-tile][128, 1536] (q | k | v)
        for m in range(NCT):
            t_ = persist.tile([P, 3 * C], BF16, tag=f"wq{m}", name=f"wq{m}")
            xq[m].dma_start(t_[:, 0:640],
                            wqkv_d.ap()[m * P:(m + 1) * P, 0:640])
            wq.append(t_)

        # biases; column m of qkb_all = qkv_b[128m:128m+128]
        qkb_all = const.tile([P, 2 * C // P], F32, tag="qkball", name="qkb_all")
        nc.gpsimd.dma_start(
            qkb_all[:], qkb_d.ap().rearrange("(m p) -> p m", p=P)
        )
        qkb_t = [qkb_all[:, m:m + 1] for m in range(2 * C // P)]

        # non-critical weight tails stay off the vector queue (it has the
        # transpose copies early) and spread over sync/scalar/gpsimd
        wq_tail_q = (nc.sync, nc.scalar, nc.gpsimd, nc.scalar)
        for m in range(NCT):
            wq_tail_q[m].dma_start(wq[m][:, 640:3 * C],
                                   wqkv_d.ap()[m * P:(m + 1) * P, 640:3 * C])
        wo = []  # [c-tile][128, 512]
        wo_q = (nc.sync, nc.scalar, nc.gpsimd, nc.sync)
        for m in range(NCT):
            t_ = persist.tile([P, C], BF16, tag=f"wo{m}", name=f"wo{m}")
            wo_q[m].dma_start(t_[:], wout_d.ap()[m * P:(m + 1) * P, :])
            wo.append(t_)

        ones8 = const.tile([P, HEADS, 1], F32, tag="ones8", name="ones8")
        nc.gpsimd.memset(ones8[:], 1.0)
        ob_row = const.tile([1, C], F32, tag="obrow", name="ob_row")
        nc.sync.dma_start(ob_row[:], ob_d.ap().rearrange("(o c) -> o c", o=1))
        obb = const.tile([P, C], F32, tag="obb", name="obb")
        nc.gpsimd.partition_broadcast(obb[:], ob_row[:], channels=P)

        VAW = HEADS * VW + (P - VW)  # 128-wide lhsT reads stay in-tile
        vaug = [persist.tile([P, VAW], BF16, tag=f"va{i}", name=f"va{i}")
                for i in range(NT)]
        qkT = [persist.tile([P, T], BF16, tag=f"qk{m}", name=f"qk{m}")
               for m in range(NCT)]
        # compact k^T per pair: head 2p in rows 0:64, head 2p+1 in 64:128
        kTc = [persist.tile([P, T], BF16, tag=f"kc{m}", name=f"kc{m}")
               for m in range(NCT)]
        anorm = [persist.tile([P, T], BF16, tag=f"an{m}", name=f"an{m}")
                 for m in range(NCT)]
        # fp32 out-projection partials (c-tiles 0-2 + bias) for t-tiles 4-7
        part = [partp.tile([P, C], F32, tag=f"pt{i}", name=f"pt{i}")
                for i in range(4)]
        # pair-0 j=0 exp slots live until its PV runs during j=1
        exh0 = [persist.tile([P, 2 * CHUNK], BF16, tag=f"ex0{g}",
                             name=f"ex0{g}") for g in range(NT)]

        # ================= background item emitters =================
        def emit_qk_half(m, j, pool):
            """qkv-projection m-tile, chunk j (q: m<4 -> qkT; k: -> kTc)."""
            ps_qk = pool.tile([P, 2 * CHUNK], F32, tag="st", name="ps_bg")
            js = slice(j * CHUNK, (j + 1) * CHUNK)
            for cc in range(NCT):
                nc.tensor.matmul(
                    ps_qk[:, 0:CHUNK],
                    wq[cc][:, m * P:(m + 1) * P],
                    xT[cc][:, js],
                    start=(cc == 0),
                    stop=(cc == NCT - 1),
                )
            dst = qkT[m] if m < NCT else kTc[m - NCT]
            nc.vector.tensor_scalar_add(dst[:, js], ps_qk[:, 0:CHUNK],
                                        qkb_t[m][:])

        def emit_v_2tiles(i0, pool):
            """vaug[i0], vaug[i0+1]: [128(t), 8, 65], [:, h, 64] = 1.0"""
            ps_v = pool.tile([P, 2 * CHUNK], F32, tag="st", name="ps_v")
            for ii in range(2):
                i = i0 + ii
                for m in range(NCT):
                    nc.tensor.matmul(
                        ps_v[:, ii * CHUNK:(ii + 1) * CHUNK],
                        xT[m][:, i * P:(i + 1) * P],
                        wq[m][:, 2 * C:3 * C],
                        start=(m == 0),
                        stop=(m == NCT - 1),
                    )
            for ii in range(2):
                i = i0 + ii
                va3 = vaug[i][:, 0:HEADS * VW].rearrange(
                    "p (h d) -> p h d", d=VW)
                nc.vector.tensor_copy(
                    va3[:, :, 0:HC],
                    ps_v[:, ii * CHUNK:(ii + 1) * CHUNK].rearrange(
                        "p (h d) -> p h d", h=HEADS),
                )
                nc.vector.tensor_copy(va3[:, :, HC:VW], ones8[:])

        def emit_outproj_partial(i, pool):
            """part[i-4] = out-proj c-tiles 0-2 + bias for t-tile i."""
            ps_o = pool.tile([P, 2 * CHUNK], F32, tag="st", name="ps_op")
            for cc in range(NCT - 1):
                nc.tensor.matmul(
                    ps_o[:, 0:CHUNK],
                    anorm[cc][:, i * P:(i + 1) * P],
                    wo[cc][:],
                    start=(cc == 0),
                    stop=(cc == NCT - 2),
                )
            nc.vector.tensor_tensor(
                part[i - 4][:], ps_o[:, 0:CHUNK], obb[:],
                op=mybir.AluOpType.add,
            )

        def emit_outproj_full(i, pool):
            """out rows [128*i, 128*(i+1)): all c-tiles + bias -> DMA."""
            ps_o = pool.tile([P, 2 * CHUNK], F32, tag="st", name="ps_of")
            for cc in range(NCT):
                nc.tensor.matmul(
                    ps_o[:, 0:CHUNK],
                    anorm[cc][:, i * P:(i + 1) * P],
                    wo[cc][:],
                    start=(cc == 0),
                    stop=(cc == NCT - 1),
                )
            ot = otp.tile([P, C], F32, tag="ot", name="ot")
            nc.vector.tensor_tensor(
                ot[:], ps_o[:, 0:CHUNK], obb[:], op=mybir.AluOpType.add,
            )
            nc.sync.dma_start(out_d.ap()[i * P:(i + 1) * P, :], ot[:])

        # ================= ramp =================
        ps1_cm = tc.tile_pool(name="ps1", bufs=2, space="PSUM")
        ps1 = ps1_cm.__enter__()
        # x PE transpose (bf16); xT[m] = x^T rows [128m,128m+128) [c, t]
        for i in range(NT):
            ps_tr = ps1.tile([P, C], BF16, tag="tr", name="ps_tr")
            for m in range(NCT):
                nc.tensor.transpose(
                    ps_tr[:, m * P:(m + 1) * P],
                    xts[i][:, m * P:(m + 1) * P],
                    identity[:],
                )
            for m in range(NCT):
                nc.vector.tensor_copy(
                    xT[m][:, i * P:(i + 1) * P], ps_tr[:, m * P:(m + 1) * P]
                )
        # only the j=0 halves of pair-0's q^T/k^T gate the first exp; their
        # j=1 halves and all of v are woven into pair-0's j=0 slots
        emit_qk_half(0, 0, ps1)             # q pair 0, t 0:512
        emit_qk_half(NCT, 0, ps1)           # k pair 0, s 0:512
        ps1_cm.__exit__(None, None, None)
        xin_cm.__exit__(None, None, None)

        # vaug junk-region zero (keeps 128-wide lhsT reads NaN-free);
        # emitted after the ramp so DVE's early queue stays clear -- only
        # needed before the first PV
        for i in range(NT):
            nc.vector.tensor_scalar_mul(
                vaug[i][:, HEADS * VW:VAW], wq[0][:, 0:VAW - HEADS * VW], 0.0
            )

        # ================= phase 2: attention (head pairs) =================
        # weave schedule: (p, j, g) -> emitter run after that slot
        WEAVE = {
            (0, 0, 0): lambda pool: emit_qk_half(0, 1, pool),
            (0, 0, 1): lambda pool: emit_qk_half(NCT, 1, pool),
            (0, 0, 2): lambda pool: emit_v_2tiles(0, pool),
            (0, 0, 3): lambda pool: emit_v_2tiles(2, pool),
            (0, 0, 4): lambda pool: emit_v_2tiles(4, pool),
            (0, 0, 5): lambda pool: emit_v_2tiles(6, pool),
            (0, 1, 1): lambda pool: emit_qk_half(1, 0, pool),
            (0, 1, 3): lambda pool: emit_qk_half(1, 1, pool),
            (0, 1, 5): lambda pool: emit_qk_half(NCT + 1, 0, pool),
            (0, 1, 6): lambda pool: emit_qk_half(NCT + 1, 1, pool),
            (1, 0, 2): lambda pool: emit_qk_half(2, 0, pool),
            (1, 0, 5): lambda pool: emit_qk_half(2, 1, pool),
            (1, 1, 2): lambda pool: emit_qk_half(NCT + 2, 0, pool),
            (1, 1, 5): lambda pool: emit_qk_half(NCT + 2, 1, pool),
            (2, 0, 2): lambda pool: emit_qk_half(3, 0, pool),
            (2, 0, 5): lambda pool: emit_qk_half(3, 1, pool),
            (2, 1, 2): lambda pool: emit_qk_half(NCT + 3, 0, pool),
            (2, 1, 5): lambda pool: emit_qk_half(NCT + 3, 1, pool),
            (3, 0, 1): lambda pool: emit_outproj_partial(4, pool),
            (3, 0, 3): lambda pool: emit_outproj_partial(5, pool),
            (3, 0, 5): lambda pool: emit_outproj_partial(6, pool),
            (3, 0, 6): lambda pool: emit_outproj_partial(7, pool),
            (3, 1, 3): lambda pool: emit_outproj_full(0, pool),
            (3, 1, 5): lambda pool: emit_outproj_full(1, pool),
            # full(2)/full(3) are emitted after the j=1 loop: their matmuls
            # have no dependency on the final normalize, so they execute
            # right behind the last PV and keep the PE warm through it
        }

        with (
            tc.tile_pool(name="expsp", bufs=6) as expsp,
            tc.tile_pool(name="ps_st", bufs=2, space="PSUM") as ps_st,
            tc.tile_pool(name="ps_pv", bufs=2, space="PSUM") as ps_pv,
        ):
            def emit_pv_pair(p, exs, pvt, g):
                """PV matmuls for s-tile g, both heads of pair p, one j."""
                for hh in range(2):
                    nc.tensor.matmul(
                        pvt[:, hh * CHUNK:(hh + 1) * CHUNK],
                        vaug[g][:, (2 * p + hh) * VW:(2 * p + hh) * VW + P],
                        exs[:, hh * CHUNK:(hh + 1) * CHUNK],
                        start=(g == 0),
                        stop=(g == NT - 1),
                    )

            def emit_normalize(p, j, pvt):
                """Scale both heads' PV by 1/denominator -> anorm[p]."""
                js = slice(j * CHUNK, (j + 1) * CHUNK)
                # NB: reciprocal_approx_fast reading PSUM directly returns
                # garbage on HW (sim-only OK) -- stage through SBUF.
                dtmp = workp.tile([1, 2 * CHUNK], F32, tag="dtmp", name="dtmp")
                recip = workp.tile([1, 2 * CHUNK], F32, tag="recip",
                                   name="recip")
                nc.vector.tensor_copy(dtmp[:], pvt[HC:HC + 1, :])
                nc.vector.reciprocal_approx_fast(recip[:], dtmp[:])
                bcast = workp.tile([HC, 2 * CHUNK], F32, tag="bcast",
                                   name="bcast")
                nc.gpsimd.partition_broadcast(bcast[:], recip[:],
                                              channels=HC)
                bsrc = bcast
                for hh in range(2):
                    nc.vector.tensor_tensor(
                        anorm[p][hh * HC:(hh + 1) * HC, js],
                        pvt[0:HC, hh * CHUNK:(hh + 1) * CHUNK],
                        bsrc[0:HC, hh * CHUNK:(hh + 1) * CHUNK],
                        op=mybir.AluOpType.mult,
                    )

            ex0slots = []
            for p in range(NPAIR):
                pvt = {}
                for j in range(NCH):
                    pvt[j] = ps_pv.tile([P, 2 * CHUNK], F32, tag="pv",
                                        name="pv")
                    exslots = []
                    for g in range(NT):
                        st_ps = ps_st.tile([P, 2 * CHUNK], F32, tag="st",
                                           name="st")
                        # two heads' S^T concurrently in row groups 0-1/2-3
                        for hh in range(2):
                            hlo = hh * HC
                            nc.tensor.matmul(
                                st_ps[:, hh * CHUNK:(hh + 1) * CHUNK],
                                kTc[p][hlo:hlo + HC, g * P:(g + 1) * P],
                                qkT[p][hlo:hlo + HC,
                                       j * CHUNK:(j + 1) * CHUNK],
                                start=True,
                                stop=True,
                            )
                        if p == 0 and j == 0:
                            exs = exh0[g]
                        else:
                            exs = expsp.tile([P, 2 * CHUNK], BF16, tag="exh",
                                             name="exh")
                        exslots.append(exs)
                        nc.scalar.activation(
                            exs[:],
                            st_ps[:],
                            mybir.ActivationFunctionType.Exp,
                            scale=EXP_SCALE,
                        )
                        # PV trails its exp by TWO slots: a lag-1 PV parks at
                        # the head of the in-order PE queue waiting on the
                        # freshest exp, idling PE every slot and knocking it
                        # off the full-speed p-state.  At lag 2 the exp is
                        # always long done, so PE never blocks.
                        if p == 0 and j == 0:
                            pass  # PV deferred to j=1 (v weaves in here)
                        elif p == 0 and j == 1:
                            emit_pv_pair(0, ex0slots[g], pvt[0], g)
                            if g >= 2:
                                emit_pv_pair(0, exslots[g - 2], pvt[1], g - 2)
                        elif g >= 2:
                            emit_pv_pair(p, exslots[g - 2], pvt[j], g - 2)
                        if (p, j, g) in WEAVE:
                            WEAVE[(p, j, g)](ps_st)
                    if p == 0 and j == 0:
                        ex0slots = exslots
                    elif p == 0 and j == 1:
                        emit_pv_pair(0, exslots[NT - 2], pvt[1], NT - 2)
                        emit_pv_pair(0, exslots[NT - 1], pvt[1], NT - 1)
                        emit_normalize(0, 0, pvt[0])
                        emit_normalize(0, 1, pvt[1])
                    else:
                        emit_pv_pair(p, exslots[NT - 2], pvt[j], NT - 2)
                        emit_pv_pair(p, exslots[NT - 1], pvt[j], NT - 1)
                        if not (p == NPAIR - 1 and j == 1):
                            emit_normalize(p, j, pvt[j])

                if p == NPAIR - 1:
                    # ---- tail: final normalize (j=1) fused with the last
                    # t-tiles' out-proj.  Quarter-pipelined: denominator
                    # copies ride the now-idle ACT engine ('copy' is in the
                    # exp table set -- no table reload), recip/mults on DVE,
                    # broadcast on GpSimd, then per t-tile c3-matmul -> add
                    # (DVE/GpSimd alternating) -> DMA.
                    emit_outproj_full(2, ps_st)
                    emit_outproj_full(3, ps_st)
                    pvl = pvt[1]
                    Q = CHUNK // 2
                    den3 = pvl[HC:HC + 1, :].rearrange(
                        "p (h q) -> p h q", h=2)
                    dts, rcs, bcs = [], [], []
                    for u in range(2):
                        dt_ = workp.tile([1, 2 * Q], F32, tag="dtmp",
                                         name=f"dtf{u}")
                        nc.scalar.activation(
                            dt_[:].rearrange("p (h q) -> p h q", h=2),
                            den3[:, :, u * Q:(u + 1) * Q],
                            mybir.ActivationFunctionType.Copy,
                        )
                        dts.append(dt_)
                    for u in range(2):
                        rc = workp.tile([1, 2 * Q], F32, tag="recip",
                                        name=f"rcf{u}")
                        nc.vector.reciprocal_approx_fast(rc[:], dts[u][:])
                        rcs.append(rc)
                        bc = workp.tile([HC, 2 * Q], F32, tag="bcast",
                                        name=f"bcf{u}")
                        nc.gpsimd.partition_broadcast(bc[:], rc[:],
                                                      channels=HC)
                        bcs.append(bc)
                    # all four mults first (they gate the c3 matmuls); the
                    # per-tile adds then stream behind them on DVE
                    for u in range(2):
                        for hh in range(2):
                            nc.vector.tensor_tensor(
                                anorm[NPAIR - 1][hh * HC:(hh + 1) * HC,
                                                 CHUNK + u * Q:
                                                 CHUNK + (u + 1) * Q],
                                pvl[0:HC, hh * CHUNK + u * Q:
                                    hh * CHUNK + (u + 1) * Q],
                                bcs[u][0:HC, hh * Q:(hh + 1) * Q],
                                op=mybir.AluOpType.mult,
                            )
                    for i in range(4, NT):
                        pool2 = (ps_pv, ps_st)[i % 2]
                        ps_o2 = pool2.tile([P, 2 * CHUNK], F32,
                                           tag="pv" if i % 2 == 0 else "st",
                                           name="ps_tl")
                        nc.tensor.matmul(
                            ps_o2[:, 0:CHUNK],
                            anorm[NCT - 1][:, i * P:(i + 1) * P],
                            wo[NCT - 1][:],
                            start=True,
                            stop=True,
                        )
                        ot = otp.tile([P, C], F32, tag="ot", name="ot")
                        nc.vector.tensor_tensor(
                            ot[:],
                            ps_o2[:, 0:CHUNK],
                            part[i - 4][:],
                            op=mybir.AluOpType.add,
                        )
                        dma_eng = (nc.sync, nc.gpsimd)[i % 2]
                        dma_eng.dma_start(
                            out_d.ap()[i * P:(i + 1) * P, :],
                            ot[:],
                        )

    nc.compile()
    return nc


def host_prep(x, qkv_w, qkv_b, out_w, out_b):
    """Host-side input prep shared by kernel() and the sim harness."""
    import ml_dtypes

    x = np.asarray(x)
    B = x.shape[0]
    x2 = x.reshape(B, T, C).astype(np.float32)
    wq2 = np.asarray(qkv_w).reshape(C, 3 * C).astype(np.float32)
    wo2 = np.asarray(out_w).reshape(C, C).astype(np.float32)
    qkv_b = np.asarray(qkv_b).astype(np.float32)
    out_b = np.asarray(out_b).astype(np.float32)

    bf = ml_dtypes.bfloat16
    x_bf = x2.astype(bf)
    wq_bf = wq2.astype(bf)
    wo_bf = wo2.astype(bf)
    # fold the v-bias through the output projection (A_norm += b_v shifts
    # out by b_v @ W_out)
    b_v = qkv_b[2 * C:3 * C]
    ob_eff = (
        out_b.astype(np.float64)
        + b_v.astype(np.float64) @ wo_bf.astype(np.float64)
    ).astype(np.float32)
    qkb = np.ascontiguousarray(qkv_b[0:2 * C])
    return x_bf, wq_bf, wo_bf, qkb, ob_eff


_CACHED_NC = None


def _get_nc():
    global _CACHED_NC
    if _CACHED_NC is None:
        _CACHED_NC = build_program()
    return _CACHED_NC


def kernel(x, qkv_w, qkv_b, out_w, out_b):
    """Full inputs in, full output out.  Shards batch across 8 NeuronCores."""
    from concourse.bass_utils import run_bass_kernel_spmd

    x = np.asarray(x)
    B, H, W, Cc = x.shape
    assert (B, H, W, Cc) == (8, 32, 32, C)
    x_bf, wq_bf, wo_bf, qkb, ob_eff = host_prep(x, qkv_w, qkv_b, out_w, out_b)

    nc = _get_nc()
    in_maps = [
        {
            "x": np.ascontiguousarray(x_bf[b]),
            "qkv_w": np.ascontiguousarray(wq_bf),
            "out_w": np.ascontiguousarray(wo_bf),
            "qk_b": qkb,
            "out_b": ob_eff,
        }
        for b in range(B)
    ]
    trace = bool(int(os.environ.get("KERNEL_TRACE", "0")))
    res = run_bass_kernel_spmd(nc, in_maps, core_ids=list(range(B)), trace=trace)
    if trace and res.exec_time_ns is not None:
        print(f"HW exec time: {res.exec_time_ns} ns")
    kernel.last_results = res
    out = np.stack([res.results[b]["out"] for b in range(B)], axis=0)
    return out.reshape(B, H, W, Cc)


kernel.last_results = None



# revision 20
# speedup vs baseline: 1.0159x; 1.0101x over previous
"""Trainium2 Bass kernel for nn_AttentionBlock (B=8, H=W=32, C=512, 8 heads).

Data-parallel over batch: each of 8 NeuronCores does one batch element.

The kernel is organized around keeping ScalarE (softmax exp, the serial
floor at ~59us of ACT work) busy while TensorE work rides in its shadow:

  ramp:    x^T loaded directly via XBAR transpose-DMA (bf16), weights
           bf16 via a second queue; q^T/k^T for head pair 0; v s-tiles
           0-1; exp-table warm-up.
  phase 2: head pairs p=0..3, j-major slots (j, g): the two heads' S^T
           s-tile matmuls run concurrently in PE row groups 0-1/2-3
           (K=64 each) -> one ScalarE exp(N=1024) -> same-pair PV
           matmuls lagging one slot.  Remaining v s-tiles, later pairs'
           q^T/k^T projections, and the entire out projection are woven
           into the S^T PSUM ring as small background items.
  tail:    per t-tile 4-7: single c-tile-3 matmul + add to the SBUF
           partial -> DMA.

All matmul operands are bf16 (fp32 PSUM accumulation, fp32 softmax
denominators/reciprocals); rel err vs the fp32 reference ~2e-3, well
inside the 2e-2 gate.  Softmax without max-subtraction (logits ~N(0,1)),
denominators via a ones-column appended to V, v-bias folded into the
output bias host-side.
"""

import math
import os
from contextlib import ExitStack

import numpy as np

import concourse.bass as bass
import concourse.mybir as mybir
import concourse.tile as tile
from concourse import bacc

T = 1024          # tokens per batch element (32*32)
C = 512           # channels
HEADS = 8
HC = C // HEADS   # 64
P = 128           # partitions
NT = T // P       # 8 t-tiles
NCT = C // P      # 4 c-tiles
CHUNK = 512       # PSUM bank = 512 fp32
NCH = T // CHUNK  # 2 chunks
F32 = mybir.dt.float32
BF16 = mybir.dt.bfloat16
EXP_SCALE = 1.0 / math.sqrt(HC)
VW = HC + 1       # v channels + ones column
NPAIR = HEADS // 2


def build_program(debug_dumps: bool = False):
    nc = bacc.Bacc("TRN2", num_devices=8, debug=False)

    x_d = nc.dram_tensor("x", [T, C], BF16, kind="ExternalInput")
    wqkv_d = nc.dram_tensor("qkv_w", [C, 3 * C], BF16, kind="ExternalInput")
    wout_d = nc.dram_tensor("out_w", [C, C], BF16, kind="ExternalInput")
    qkb_d = nc.dram_tensor("qk_b", [2 * C], F32, kind="ExternalInput")
    ob_d = nc.dram_tensor("out_b", [C], F32, kind="ExternalInput")
    out_d = nc.dram_tensor("out", [T, C], F32, kind="ExternalOutput")

    with tile.TileContext(nc) as tc, ExitStack() as ctx:
        # ---------------- SBUF pools ----------------
        const = ctx.enter_context(tc.tile_pool(name="const", bufs=1))
        persist = ctx.enter_context(tc.tile_pool(name="persist", bufs=1))
        workp = ctx.enter_context(tc.tile_pool(name="workp", bufs=6))
        otp = ctx.enter_context(tc.tile_pool(name="otp", bufs=4))
        partp = ctx.enter_context(tc.tile_pool(name="partp", bufs=1))

        # exp table warm-up ASAP (the ~2.7us table load hides under DMA)
        warm = const.tile([1, 2], F32, tag="warm", name="warm")
        nc.gpsimd.memset(warm[:, 0:1], 0.0)
        nc.scalar.activation(warm[:, 1:2], warm[:, 0:1],
                             mybir.ActivationFunctionType.Exp)

        # x tiles then PE transposes (XBAR transpose-DMA measured ~8us/tile
        # -- far slower than the PE path).  x^T lives in ONE [128, 4, T]
        # tile so each t-tile's PSUM drain is a single strided copy.
        xTall = persist.tile([P, NCT, T], BF16, tag="xTall", name="xTall")
        xT = [xTall[:, m, :] for m in range(NCT)]
        identity = const.tile([P, P], BF16, tag="ident", name="ident")
        from concourse.masks import make_identity
        make_identity(nc, identity[:])
        xin_cm = tc.tile_pool(name="xin", bufs=1)
        xin = xin_cm.__enter__()
        # DMA priority: xpair0/1 (t 0:512) and the four wq[0:640] chunks
        # gate the first S^T slot; everything else queues behind them.
        # Queues: sync(SP) / scalar(ACT) / gpsimd(SWDGE).
        xpq = (nc.sync, nc.scalar, nc.gpsimd, nc.sync)
        xpair = []
        for ip in range(NT // 2):
            xt_in = xin.tile([P, 2 * C], BF16, tag=f"xin{ip}", name=f"xin{ip}")
            xpair.append(xt_in)

        def xpair_dma(ip):
            xpq[ip].dma_start(
                xpair[ip][:].rearrange("p (u c) -> p u c", u=2),
                x_d.ap()[2 * ip * P:(2 * ip + 2) * P, :]
                .rearrange("(u p) c -> p u c", p=P))

        wq = [persist.tile([P, 3 * C], BF16, tag=f"wq{m}", name=f"wq{m}")
              for m in range(NCT)]
        wqaq = (nc.sync, nc.scalar, nc.gpsimd, nc.gpsimd)
        xpair_dma(0)                       # sync
        xpair_dma(1)                       # scalar
        for m in range(NCT):               # critical wq cols 0:640
            wqaq[m].dma_start(wq[m][:, 0:640],
                              wqkv_d.ap()[m * P:(m + 1) * P, 0:640])
        xpair_dma(2)                       # gpsimd
        xpair_dma(3)                       # sync
        xts = [xpair[i // 2][:, (i % 2) * C:(i % 2 + 1) * C]
               for i in range(NT)]

        # biases; column m of qkb_all = qkv_b[128m:128m+128]
        qkb_all = const.tile([P, 2 * C // P], F32, tag="qkball", name="qkb_all")
        nc.gpsimd.dma_start(
            qkb_all[:], qkb_d.ap().rearrange("(m p) -> p m", p=P)
        )
        qkb_t = [qkb_all[:, m:m + 1] for m in range(2 * C // P)]

        wq_tail_q = (nc.sync, nc.scalar, nc.gpsimd, nc.scalar)
        for m in range(NCT):
            wq_tail_q[m].dma_start(wq[m][:, 640:3 * C],
                                   wqkv_d.ap()[m * P:(m + 1) * P, 640:3 * C])
        wo = []  # [c-tile][128, 512]
        wo_q = (nc.sync, nc.scalar, nc.gpsimd, nc.sync)
        for m in range(NCT):
            t_ = persist.tile([P, C], BF16, tag=f"wo{m}", name=f"wo{m}")
            wo_q[m].dma_start(t_[:], wout_d.ap()[m * P:(m + 1) * P, :])
            wo.append(t_)

        ones8 = const.tile([P, HEADS, 1], F32, tag="ones8", name="ones8")
        nc.gpsimd.memset(ones8[:], 1.0)
        ob_row = const.tile([1, C], F32, tag="obrow", name="ob_row")
        nc.sync.dma_start(ob_row[:], ob_d.ap().rearrange("(o c) -> o c", o=1))
        obb = const.tile([P, C], F32, tag="obb", name="obb")
        nc.gpsimd.partition_broadcast(obb[:], ob_row[:], channels=P)

        VAW = HEADS * VW + (P - VW)  # 128-wide lhsT reads stay in-tile
        vaug = [persist.tile([P, VAW], BF16, tag=f"va{i}", name=f"va{i}")
                for i in range(NT)]
        qkT = [persist.tile([P, T], BF16, tag=f"qk{m}", name=f"qk{m}")
               for m in range(NCT)]
        # compact k^T per pair: head 2p in rows 0:64, head 2p+1 in 64:128
        kTc = [persist.tile([P, T], BF16, tag=f"kc{m}", name=f"kc{m}")
               for m in range(NCT)]
        anorm = [persist.tile([P, T], BF16, tag=f"an{m}", name=f"an{m}")
                 for m in range(NCT)]
        # fp32 out-projection partials (c-tiles 0-2 + bias) for t-tiles 4-7
        part = [partp.tile([P, C], F32, tag=f"pt{i}", name=f"pt{i}")
                for i in range(4)]
        # pair-0 j=0 exp slots live until its PV runs during j=1
        exh0 = [persist.tile([P, 2 * CHUNK], BF16, tag=f"ex0{g}",
                             name=f"ex0{g}") for g in range(NT)]

        # ================= background item emitters =================
        def emit_qk_half(m, j, pool):
            """qkv-projection m-tile, chunk j (q: m<4 -> qkT; k: -> kTc)."""
            ps_qk = pool.tile([P, 2 * CHUNK], F32, tag="st", name="ps_bg")
            js = slice(j * CHUNK, (j + 1) * CHUNK)
            for cc in range(NCT):
                nc.tensor.matmul(
                    ps_qk[:, 0:CHUNK],
                    wq[cc][:, m * P:(m + 1) * P],
                    xT[cc][:, js],
                    start=(cc == 0),
                    stop=(cc == NCT - 1),
                )
            dst = qkT[m] if m < NCT else kTc[m - NCT]
            nc.vector.tensor_scalar_add(dst[:, js], ps_qk[:, 0:CHUNK],
                                        qkb_t[m][:])

        def emit_v_2tiles(i0, pool):
            """vaug[i0], vaug[i0+1]: [128(t), 8, 65], [:, h, 64] = 1.0"""
            ps_v = pool.tile([P, 2 * CHUNK], F32, tag="st", name="ps_v")
            for ii in range(2):
                i = i0 + ii
                for m in range(NCT):
                    nc.tensor.matmul(
                        ps_v[:, ii * CHUNK:(ii + 1) * CHUNK],
                        xT[m][:, i * P:(i + 1) * P],
                        wq[m][:, 2 * C:3 * C],
                        start=(m == 0),
                        stop=(m == NCT - 1),
                    )
            for ii in range(2):
                i = i0 + ii
                va3 = vaug[i][:, 0:HEADS * VW].rearrange(
                    "p (h d) -> p h d", d=VW)
                nc.vector.tensor_copy(
                    va3[:, :, 0:HC],
                    ps_v[:, ii * CHUNK:(ii + 1) * CHUNK].rearrange(
                        "p (h d) -> p h d", h=HEADS),
                )
                nc.vector.tensor_copy(va3[:, :, HC:VW], ones8[:])

        def emit_outproj_partial(i, pool):
            """part[i-4] = out-proj c-tiles 0-2 + bias for t-tile i."""
            ps_o = pool.tile([P, 2 * CHUNK], F32, tag="st", name="ps_op")
            for cc in range(NCT - 1):
                nc.tensor.matmul(
                    ps_o[:, 0:CHUNK],
                    anorm[cc][:, i * P:(i + 1) * P],
                    wo[cc][:],
                    start=(cc == 0),
                    stop=(cc == NCT - 2),
                )
            nc.vector.tensor_tensor(
                part[i - 4][:], ps_o[:, 0:CHUNK], obb[:],
                op=mybir.AluOpType.add,
            )

        def emit_outproj_full(i, pool):
            """out rows [128*i, 128*(i+1)): all c-tiles + bias -> DMA."""
            ps_o = pool.tile([P, 2 * CHUNK], F32, tag="st", name="ps_of")
            for cc in range(NCT):
                nc.tensor.matmul(
                    ps_o[:, 0:CHUNK],
                    anorm[cc][:, i * P:(i + 1) * P],
                    wo[cc][:],
                    start=(cc == 0),
                    stop=(cc == NCT - 1),
                )
            ot = otp.tile([P, C], F32, tag="ot", name="ot")
            nc.vector.tensor_tensor(
                ot[:], ps_o[:, 0:CHUNK], obb[:], op=mybir.AluOpType.add,
            )
            nc.sync.dma_start(out_d.ap()[i * P:(i + 1) * P, :], ot[:])

        # ================= ramp =================
        ps1_cm = tc.tile_pool(name="ps1", bufs=2, space="PSUM")
        ps1 = ps1_cm.__enter__()
        # x PE transpose (bf16); xT[m] = x^T rows [128m,128m+128) [c, t].
        # t-tiles 0-3 first, then the pair-0 q/k projections (which only
        # need t/s 0:512), THEN t-tiles 4-7 -- keeps the late x pairs off
        # the first-exp critical path.
        def emit_tr(i):
            ps_tr = ps1.tile([P, C], BF16, tag="tr", name="ps_tr")
            for m in range(NCT):
                nc.tensor.transpose(
                    ps_tr[:, m * P:(m + 1) * P],
                    xts[i][:, m * P:(m + 1) * P],
                    identity[:],
                )
            nc.vector.tensor_copy(
                xTall[:, :, i * P:(i + 1) * P],
                ps_tr[:].rearrange("p (m q) -> p m q", m=NCT),
            )

        for i in range(NT // 2):
            emit_tr(i)
        # only the j=0 halves of pair-0's q^T/k^T gate the first exp; their
        # j=1 halves and all of v are woven into pair-0's j=0 slots
        emit_qk_half(0, 0, ps1)             # q pair 0, t 0:512
        emit_qk_half(NCT, 0, ps1)           # k pair 0, s 0:512
        for i in range(NT // 2, NT):
            emit_tr(i)
        ps1_cm.__exit__(None, None, None)
        xin_cm.__exit__(None, None, None)

        # vaug junk-region zero (keeps 128-wide lhsT reads NaN-free);
        # emitted after the ramp so DVE's early queue stays clear -- only
        # needed before the first PV
        for i in range(NT):
            nc.vector.tensor_scalar_mul(
                vaug[i][:, HEADS * VW:VAW], wq[0][:, 0:VAW - HEADS * VW], 0.0
            )

        # ================= phase 2: attention (head pairs) =================
        # weave schedule: (p, j, g) -> emitter run after that slot
        WEAVE = {
            (0, 0, 0): lambda pool: emit_qk_half(0, 1, pool),
            (0, 0, 1): lambda pool: emit_qk_half(NCT, 1, pool),
            (0, 0, 2): lambda pool: emit_v_2tiles(0, pool),
            (0, 0, 3): lambda pool: emit_v_2tiles(2, pool),
            (0, 0, 4): lambda pool: emit_v_2tiles(4, pool),
            (0, 0, 5): lambda pool: emit_v_2tiles(6, pool),
            (0, 1, 1): lambda pool: emit_qk_half(1, 0, pool),
            (0, 1, 3): lambda pool: emit_qk_half(1, 1, pool),
            (0, 1, 5): lambda pool: emit_qk_half(NCT + 1, 0, pool),
            (0, 1, 6): lambda pool: emit_qk_half(NCT + 1, 1, pool),
            (1, 0, 2): lambda pool: emit_qk_half(2, 0, pool),
            (1, 0, 5): lambda pool: emit_qk_half(2, 1, pool),
            (1, 1, 2): lambda pool: emit_qk_half(NCT + 2, 0, pool),
            (1, 1, 5): lambda pool: emit_qk_half(NCT + 2, 1, pool),
            (2, 0, 2): lambda pool: emit_qk_half(3, 0, pool),
            (2, 0, 5): lambda pool: emit_qk_half(3, 1, pool),
            (2, 1, 2): lambda pool: emit_qk_half(NCT + 3, 0, pool),
            (2, 1, 5): lambda pool: emit_qk_half(NCT + 3, 1, pool),
            (3, 0, 2): lambda pool: emit_outproj_partial(4, pool),
            (3, 0, 4): lambda pool: emit_outproj_partial(5, pool),
            (3, 0, 6): lambda pool: emit_outproj_partial(6, pool),
            (3, 0, 7): lambda pool: emit_outproj_partial(7, pool),
            (3, 1, 3): lambda pool: emit_outproj_full(0, pool),
            (3, 1, 5): lambda pool: emit_outproj_full(1, pool),
            # full(2)/full(3) are emitted after the j=1 loop: their matmuls
            # have no dependency on the final normalize, so they execute
            # right behind the last PV and keep the PE warm through it
        }

        with (
            tc.tile_pool(name="expsp", bufs=6) as expsp,
            tc.tile_pool(name="ps_st", bufs=2, space="PSUM") as ps_st,
            tc.tile_pool(name="ps_pv", bufs=2, space="PSUM") as ps_pv,
        ):
            def emit_pv_pair(p, exs, pvt, g):
                """PV matmuls for s-tile g, both heads of pair p, one j."""
                for hh in range(2):
                    nc.tensor.matmul(
                        pvt[:, hh * CHUNK:(hh + 1) * CHUNK],
                        vaug[g][:, (2 * p + hh) * VW:(2 * p + hh) * VW + P],
                        exs[:, hh * CHUNK:(hh + 1) * CHUNK],
                        start=(g == 0),
                        stop=(g == NT - 1),
                    )

            def emit_normalize(p, j, pvt):
                """Scale both heads' PV by 1/denominator -> anorm[p]."""
                js = slice(j * CHUNK, (j + 1) * CHUNK)
                # NB: reciprocal_approx_fast reading PSUM directly returns
                # garbage on HW (sim-only OK) -- stage through SBUF.
                dtmp = workp.tile([1, 2 * CHUNK], F32, tag="dtmp", name="dtmp")
                recip = workp.tile([1, 2 * CHUNK], F32, tag="recip",
                                   name="recip")
                nc.vector.tensor_copy(dtmp[:], pvt[HC:HC + 1, :])
                nc.vector.reciprocal_approx_fast(recip[:], dtmp[:])
                bcast = workp.tile([HC, 2 * CHUNK], F32, tag="bcast",
                                   name="bcast")
                nc.gpsimd.partition_broadcast(bcast[:], recip[:],
                                              channels=HC)
                bsrc = bcast
                for hh in range(2):
                    nc.vector.tensor_tensor(
                        anorm[p][hh * HC:(hh + 1) * HC, js],
                        pvt[0:HC, hh * CHUNK:(hh + 1) * CHUNK],
                        bsrc[0:HC, hh * CHUNK:(hh + 1) * CHUNK],
                        op=mybir.AluOpType.mult,
                    )

            # Global PV pipeline: PVs trail their exp by two slots AND ride
            # across j/p boundaries, so the PE never parks at the queue
            # head waiting on a fresh exp (which would also knock it off
            # the full-speed p-state).  Each (p, j)'s normalize is emitted
            # right after its last PV drains.
            pending = []

            def drain_pv(limit):
                while len(pending) > limit:
                    pp, exs_, pvtile, g_, norm = pending.pop(0)
                    emit_pv_pair(pp, exs_, pvtile, g_)
                    if norm is not None:
                        emit_normalize(*norm)

            ex0slots = []
            for p in range(NPAIR):
                pvt = {}
                for j in range(NCH):
                    pvt[j] = ps_pv.tile([P, 2 * CHUNK], F32, tag="pv",
                                        name="pv")
                    exslots = []
                    for g in range(NT):
                        st_ps = ps_st.tile([P, 2 * CHUNK], F32, tag="st",
                                           name="st")
                        # two heads' S^T concurrently in row groups 0-1/2-3
                        for hh in range(2):
                            hlo = hh * HC
                            nc.tensor.matmul(
                                st_ps[:, hh * CHUNK:(hh + 1) * CHUNK],
                                kTc[p][hlo:hlo + HC, g * P:(g + 1) * P],
                                qkT[p][hlo:hlo + HC,
                                       j * CHUNK:(j + 1) * CHUNK],
                                start=True,
                                stop=True,
                            )
                        if p == 0 and j == 0:
                            exs = exh0[g]
                        else:
                            exs = expsp.tile([P, 2 * CHUNK], BF16, tag="exh",
                                             name="exh")
                        exslots.append(exs)
                        nc.scalar.activation(
                            exs[:],
                            st_ps[:],
                            mybir.ActivationFunctionType.Exp,
                            scale=EXP_SCALE,
                        )
                        if p == 0 and j == 0:
                            pass  # PV deferred to j=1 (v weaves in here)
                        else:
                            if p == 0 and j == 1:
                                # pair-0 j=0 PVs: exps are ancient, lag 0
                                emit_pv_pair(0, ex0slots[g], pvt[0], g)
                                if g == NT - 1:
                                    emit_normalize(0, 0, pvt[0])
                            norm = None
                            if g == NT - 1 and not (p == NPAIR - 1
                                                    and j == 1):
                                norm = (p, j, pvt[j])
                            pending.append((p, exs, pvt[j], g, norm))
                            drain_pv(2)
                        if (p, j, g) in WEAVE:
                            WEAVE[(p, j, g)](ps_st)
                    if p == 0 and j == 0:
                        ex0slots = exslots
                    else:
                        # at the turn of a j, let the last slot's PV ride
                        # into the next j's stream (lag 1)
                        drain_pv(1)

                if p == NPAIR - 1:
                    drain_pv(0)
                    # ---- tail: final normalize (j=1) fused with the last
                    # t-tiles' out-proj.  Quarter-pipelined: denominator
                    # copies ride the now-idle ACT engine ('copy' is in the
                    # exp table set -- no table reload), recip/mults on DVE,
                    # broadcast on GpSimd, then per t-tile c3-matmul -> add
                    # (DVE/GpSimd alternating) -> DMA.
                    emit_outproj_full(2, ps_st)
                    emit_outproj_full(3, ps_st)
                    pvl = pvt[1]
                    Q = CHUNK // 2
                    den3 = pvl[HC:HC + 1, :].rearrange(
                        "p (h q) -> p h q", h=2)
                    dts, rcs, bcs = [], [], []
                    for u in range(2):
                        dt_ = workp.tile([1, 2 * Q], F32, tag="dtmp",
                                         name=f"dtf{u}")
                        nc.scalar.activation(
                            dt_[:].rearrange("p (h q) -> p h q", h=2),
                            den3[:, :, u * Q:(u + 1) * Q],
                            mybir.ActivationFunctionType.Copy,
                        )
                        dts.append(dt_)
                    for u in range(2):
                        rc = workp.tile([1, 2 * Q], F32, tag="recip",
                                        name=f"rcf{u}")
                        nc.vector.reciprocal_approx_fast(rc[:], dts[u][:])
                        rcs.append(rc)
                        bc = workp.tile([HC, 2 * Q], F32, tag="bcast",
                                        name=f"bcf{u}")
                        nc.gpsimd.partition_broadcast(bc[:], rc[:],
                                                      channels=HC)
                        bcs.append(bc)
                    # all four mults first (they gate the c3 matmuls); the
                    # per-tile adds then stream behind them on DVE
                    for u in range(2):
                        for hh in range(2):
                            nc.vector.tensor_tensor(
                                anorm[NPAIR - 1][hh * HC:(hh + 1) * HC,
                                                 CHUNK + u * Q:
                                                 CHUNK + (u + 1) * Q],
                                pvl[0:HC, hh * CHUNK + u * Q:
                                    hh * CHUNK + (u + 1) * Q],
                                bcs[u][0:HC, hh * Q:(hh + 1) * Q],
                                op=mybir.AluOpType.mult,
                            )
                    for i in range(4, NT):
                        pool2 = (ps_pv, ps_st)[i % 2]
                        ps_o2 = pool2.tile([P, 2 * CHUNK], F32,
                                           tag="pv" if i % 2 == 0 else "st",
                                           name="ps_tl")
                        nc.tensor.matmul(
                            ps_o2[:, 0:CHUNK],
                            anorm[NCT - 1][:, i * P:(i + 1) * P],
                            wo[NCT - 1][:],
                            start=True,
                            stop=True,
                        )
                        ot = otp.tile([P, C], F32, tag="ot", name="ot")
                        nc.vector.tensor_tensor(
                            ot[:],
                            ps_o2[:, 0:CHUNK],
                            part[i - 4][:],
                            op=mybir.AluOpType.add,
                        )
                        dma_eng = (nc.sync, nc.gpsimd)[i % 2]
                        dma_eng.dma_start(
                            out_d.ap()[i * P:(i + 1) * P, :],
                            ot[:],
                        )

    nc.compile()
    return nc


def host_prep(x, qkv_w, qkv_b, out_w, out_b):
    """Host-side input prep shared by kernel() and the sim harness."""
    import ml_dtypes

    x = np.asarray(x)
    B = x.shape[0]
    x2 = x.reshape(B, T, C).astype(np.float32)
    wq2 = np.asarray(qkv_w).reshape(C, 3 * C).astype(np.float32)
    wo2 = np.asarray(out_w).reshape(C, C).astype(np.float32)
    qkv_b = np.asarray(qkv_b).astype(np.float32)
    out_b = np.asarray(out_b).astype(np.float32)

    bf = ml_dtypes.bfloat16
    x_bf = x2.astype(bf)
    wq_bf = wq2.astype(bf)
    wo_bf = wo2.astype(bf)
    # fold the v-bias through the output projection (A_norm += b_v shifts
    # out by b_v @ W_out)
    b_v = qkv_b[2 * C:3 * C]
    ob_eff = (
        out_b.astype(np.float64)
        + b_v.astype(np.float64) @ wo_bf.astype(np.float64)
    ).astype(np.float32)
    qkb = np.ascontiguousarray(qkv_b[0:2 * C])
    return x_bf, wq_bf, wo_bf, qkb, ob_eff


_CACHED_NC = None


def _get_nc():
    global _CACHED_NC
    if _CACHED_NC is None:
        _CACHED_NC = build_program()
    return _CACHED_NC


def kernel(x, qkv_w, qkv_b, out_w, out_b):
    """Full inputs in, full output out.  Shards batch across 8 NeuronCores."""
    from concourse.bass_utils import run_bass_kernel_spmd

    x = np.asarray(x)
    B, H, W, Cc = x.shape
    assert (B, H, W, Cc) == (8, 32, 32, C)
    x_bf, wq_bf, wo_bf, qkb, ob_eff = host_prep(x, qkv_w, qkv_b, out_w, out_b)

    nc = _get_nc()
    in_maps = [
        {
            "x": np.ascontiguousarray(x_bf[b]),
            "qkv_w": np.ascontiguousarray(wq_bf),
            "out_w": np.ascontiguousarray(wo_bf),
            "qk_b": qkb,
            "out_b": ob_eff,
        }
        for b in range(B)
    ]
    trace = bool(int(os.environ.get("KERNEL_TRACE", "0")))
    res = run_bass_kernel_spmd(nc, in_maps, core_ids=list(range(B)), trace=trace)
    if trace and res.exec_time_ns is not None:
        print(f"HW exec time: {res.exec_time_ns} ns")
    kernel.last_results = res
    out = np.stack([res.results[b]["out"] for b in range(B)], axis=0)
    return out.reshape(B, H, W, Cc)


kernel.last_results = None

